# revision 11
# baseline (speedup 1.0000x reference)
"""ActiveShiftLayer Trainium2 kernel.

out[n,c,h,w] = bilinear sample of x[n,c, h+alpha_c, w+beta_c], zero outside.

alpha,beta in [-1,1) => floor in {-1,0}; the bilinear sample is a separable
3-tap convolution along W then H with per-channel tap weights:
    tmp[h,w] = sum_dx wh[c,dx] * x[h, w+dx]      (dx in {-1,0,1}, zero pad)
    out[h,w] = sum_dy wv[c,dy] * tmp[h+dy, w]    (dy in {-1,0,1}, zero pad)
Weights are computed on host from shift_param [C,2] and passed as extra
inputs.

Data-parallel over batch (N=32 -> 4 per core); per core 8 tiles of
[128 channels (partitions), 56*56 plane (free dim)].

Per-tile schedule (f32 end-to-end except the H-stage products in float32r):
- contiguous DMA load into X[128, 1+3136+1] (1-elem guard pads)
- H-stage on TensorE: per 512-col chunk, 3 accumulating float32r matmuls
  with diagonal weight matrices (diag applies per-channel tap weight); flat
  taps at offsets {-1,0,+1} wrap across row boundaries, fixed later
- ScalarE copies PSUM -> SBUF HT center (rows 1..56 of a 58-row buffer
  whose first/last rows are zeroed)
- GPSIMD fixes the two wrapped columns: tmp[h][0] -= wh_m1*x[h-1][55],
  tmp[h][55] -= wh_p1*x[h+1][0] (strided 56-elem scalar_tensor_tensor)
- V-stage: ScalarE center tap (activation scale), VectorE outer taps
  (scalar_tensor_tensor accumulate), all exact f32
- contiguous DMA store
"""

import os
import numpy as np

N, C, H, W = 32, 256, 56, 56
NCORES = 8
NSH = N // NCORES  # batches per core
P = 128
CB = C // P        # channel blocks
HW = H * W         # 3136
XF = HW + 2        # X tile free size (guard pad at 0 and HW+1)
HTF = (H + 2) * W  # 3248
CHUNK = 512
NCHUNK = (HW + CHUNK - 1) // CHUNK  # 7 (last chunk = 64)

_CACHE = {}


def _build_nc():
    import concourse.bacc as bacc
    import concourse.mybir as mybir
    import concourse.tile as tile

    f32 = mybir.dt.float32
    f32r = mybir.dt.float32r
    mult = mybir.AluOpType.mult
    add = mybir.AluOpType.add
    act_copy = mybir.ActivationFunctionType.Copy

    nc = bacc.Bacc()
    xs = nc.dram_tensor("xs", [NSH, C, H, W], f32, kind="ExternalInput")
    # wd[cb, tap] = diag(wh_tap) for channels cb*128..cb*128+127
    wd = nc.dram_tensor("wd", [CB, 3, P, P], f32, kind="ExternalInput")
    # wv[cb] columns: [wv_m1, wv_0, wv_p1, -wh_m1, -wh_p1]
    wv = nc.dram_tensor("wv", [CB, P, 5], f32, kind="ExternalInput")
    ys = nc.dram_tensor("ys", [NSH, C, H, W], f32, kind="ExternalOutput")

    with tile.TileContext(nc) as tc:
        with tc.tile_pool(name="wp", bufs=1) as wp, \
             tc.tile_pool(name="xp", bufs=3) as xpool, \
             tc.tile_pool(name="ht", bufs=3) as hpool, \
             tc.tile_pool(name="op", bufs=3) as opool, \
             tc.tile_pool(name="ps", bufs=1, space="PSUM") as ppool:

            wdt = []
            wvt = []
            for cb in range(CB):
                t = wp.tile([P, 3 * P], f32r, tag=f"wd{cb}")
                nc.gpsimd.dma_start(
                    t[:].rearrange("p (t q) -> p t q", t=3),
                    wd[cb].rearrange("t p q -> p t q"))
                wdt.append(t)
                v = wp.tile([P, 5], f32, tag=f"wv{cb}")
                nc.sync.dma_start(v[:], wv[cb])
                wvt.append(v)

            for n in range(NSH):
                for cb in range(CB):
                    wvc = wvt[cb]
                    cs = slice(cb * P, (cb + 1) * P)

                    X = xpool.tile([P, XF], f32r)
                    nc.gpsimd.memset(X[:, 0:1].bitcast(f32), 0.0)
                    nc.gpsimd.memset(X[:, XF - 1:XF].bitcast(f32), 0.0)
                    nc.gpsimd.dma_start(X[:, 1:1 + HW], xs[n, cs, :, :])

                    PS = ppool.tile([P, HW], f32, tag="ps")
                    for j in range(NCHUNK):
                        c0 = j * CHUNK
                        cn = min(CHUNK, HW - c0)
                        for tap in range(3):
                            nc.tensor.matmul(
                                PS[:, c0:c0 + cn],
                                wdt[cb][:, tap * P:(tap + 1) * P],
                                X[:, c0 + tap:c0 + tap + cn],
                                start=(tap == 0), stop=(tap == 2))

                    HT = hpool.tile([P, HTF], f32)
                    nc.gpsimd.memset(HT[:, 0:W], 0.0)
                    nc.gpsimd.memset(HT[:, HTF - W:], 0.0)
                    ctr = HT[:, W:W + HW]
                    # chunked PSUM->SBUF copies release PSUM banks
                    # incrementally so the next tile's matmuls can start
                    for j in range(NCHUNK):
                        c0 = j * CHUNK
                        cn = min(CHUNK, HW - c0)
                        nc.scalar.activation(
                            HT[:, W + c0:W + c0 + cn], PS[:, c0:c0 + cn],
                            act_copy)

    # wrapped-column fixups (strided, per channel):
                    #   tmp[h][0]  -= wh_m1 * x[h-1][55]   (x[-1][55] := guard X[0] = 0)
                    #   tmp[h][55] -= wh_p1 * x[h+1][0]    (h=55 term is 0: guard X[3137])
                    ctr2 = ctr.rearrange("p (h w) -> p h w", w=W)
                    col0 = ctr2[:, :, 0]
                    col55 = ctr2[:, 0:H - 1, W - 1]
                    # x[h-1][55] = X[56h]; x[h+1][0] = X[1 + 56(h+1)]
                    xg0 = X[:, 0:HW].bitcast(f32).rearrange(
                        "p (h w) -> p h w", w=W)[:, :, 0]
                    xg55 = X[:, 1:1 + HW].bitcast(f32).rearrange(
                        "p (h w) -> p h w", w=W)[:, 1:H, 0]
                    nc.vector.scalar_tensor_tensor(
                        col0, xg0, wvc[:, 3:4], col0, op0=mult, op1=add)
                    nc.vector.scalar_tensor_tensor(
                        col55, xg55, wvc[:, 4:5], col55, op0=mult, op1=add)

                    OUT = opool.tile([P, HW], f32)
                    nc.scalar.activation(OUT[:], ctr, act_copy, scale=wvc[:, 1:2])
                    nc.vector.scalar_tensor_tensor(
                        OUT[:], HT[:, 0:HW], wvc[:, 0:1], OUT[:], op0=mult, op1=add)
                    nc.vector.scalar_tensor_tensor(
                        OUT[:], HT[:, 2 * W:2 * W + HW], wvc[:, 2:3], OUT[:],
                        op0=mult, op1=add)

                    nc.sync.dma_start(
                        ys[n, cs, :, :],
                        OUT[:].rearrange("p (h w) -> p h w", w=W))
    nc.finalize()
    return nc


def _tap_weights(shift):
    """Per-channel 3-tap weights over offsets {-1,0,1} for shift in [-1,1)."""
    f = np.floor(shift)
    t = (shift - f).astype(np.float32)
    assert np.all((f == -1) | (f == 0)), "shift outside [-1,1) unsupported"
    w_m1 = np.where(f == -1, 1 - t, 0).astype(np.float32)
    w_0 = np.where(f == -1, t, 1 - t).astype(np.float32)
    w_p1 = np.where(f == 0, t, 0).astype(np.float32)
    return w_m1, w_0, w_p1


def _host_weights(sp):
    wh_m1, wh_0, wh_p1 = _tap_weights(sp[:, 1])  # beta: W shift
    wv_m1, wv_0, wv_p1 = _tap_weights(sp[:, 0])  # alpha: H shift
    wd = np.zeros((CB, 3, P, P), np.float32)
    for cb in range(CB):
        cs = slice(cb * P, (cb + 1) * P)
        for t, w in enumerate((wh_m1, wh_0, wh_p1)):
            wd[cb, t] = np.diag(w[cs])
    wv = np.stack([wv_m1, wv_0, wv_p1, -wh_m1, -wh_p1], axis=1).astype(np.float32)
    wv = np.ascontiguousarray(wv.reshape(CB, P, 5))
    return np.ascontiguousarray(wd), wv


def _install_trace_shim():
    """Dev-only: register the NTFF profile hook this container's antenv lacks,
    and stub out the artifact upload (zero-egress container)."""
    import sys
    import types

    try:
        from antenv.axon_hooks import get_axon_ntff_profile_hook  # noqa: F401
    except ImportError:
        from trn_agent_boot.trn_boot import _ntff_profile_via_ctypes

        hook = _ntff_profile_via_ctypes("/opt/axon/libaxon_pjrt.so")
        mod = types.ModuleType("antenv.axon_hooks")
        mod.get_axon_ntff_profile_hook = lambda: hook
        mod.set_axon_ntff_profile_hook = lambda h: None
        import antenv

        sys.modules["antenv.axon_hooks"] = mod
        antenv.axon_hooks = mod

    import concourse.bass_utils as bu

    bu.upload_artifacts = lambda tmpdir: tmpdir


def kernel(x, shift_param):
    from concourse.bass_utils import run_bass_kernel_spmd

    x = np.ascontiguousarray(np.asarray(x, dtype=np.float32))
    sp = np.asarray(shift_param, dtype=np.float32)
    assert x.shape == (N, C, H, W)

    wd, wv = _host_weights(sp)

    if "nc" not in _CACHE:
        _CACHE["nc"] = _build_nc()
    nc = _CACHE["nc"]

    in_maps = [{"xs": x[i * NSH:(i + 1) * NSH], "wd": wd, "wv": wv}
               for i in range(NCORES)]
    trace = os.environ.get("ASL_TRACE") == "1"
    if trace:
        _install_trace_shim()
    res = run_bass_kernel_spmd(nc, in_maps, list(range(NCORES)), trace=trace)
    if trace:
        print(f"HW exec time: {res.exec_time_ns} ns")
        _CACHE["last_result"] = res
    out = np.concatenate([r["ys"] for r in res.results], axis=0)
    return out


# revision 13
# speedup vs baseline: 1.0832x; 1.0832x over previous
"""ActiveShiftLayer Trainium2 kernel.

out[n,c,h,w] = bilinear sample of x[n,c, h+alpha_c, w+beta_c], zero outside.

alpha,beta in [-1,1) => floor in {-1,0}; the bilinear sample is a separable
3-tap convolution along W then H with per-channel tap weights:
    tmp[h,w] = sum_dx wh[c,dx] * x[h, w+dx]      (dx in {-1,0,1}, zero pad)
    out[h,w] = sum_dy wv[c,dy] * tmp[h+dy, w]    (dy in {-1,0,1}, zero pad)
Weights are computed on host from shift_param [C,2] and passed as extra
inputs.

Data-parallel over batch (N=32 -> 4 per core); per core 8 tiles of
[128 channels (partitions), 56*56 plane (free dim)].

Per-tile schedule (f32 end-to-end except the H-stage products in float32r):
- contiguous DMA load into X[128, 1+3136+1] (1-elem guard pads)
- H-stage on TensorE: per 512-col chunk, 3 accumulating float32r matmuls
  with diagonal weight matrices (diag applies per-channel tap weight); flat
  taps at offsets {-1,0,+1} wrap across row boundaries, fixed later
- ScalarE copies PSUM -> SBUF HT center (rows 1..56 of a 58-row buffer
  whose first/last rows are zeroed)
- GPSIMD fixes the two wrapped columns: tmp[h][0] -= wh_m1*x[h-1][55],
  tmp[h][55] -= wh_p1*x[h+1][0] (strided 56-elem scalar_tensor_tensor)
- V-stage: ScalarE center tap (activation scale), VectorE outer taps
  (scalar_tensor_tensor accumulate), all exact f32
- contiguous DMA store
"""

import os
import numpy as np

N, C, H, W = 32, 256, 56, 56
NCORES = 8
NSH = N // NCORES  # batches per core
P = 128
CB = C // P        # channel blocks
HW = H * W         # 3136
XF = HW + 2        # X tile free size (guard pad at 0 and HW+1)
HTF = (H + 2) * W  # 3248
CHUNK = 512
NCHUNK = (HW + CHUNK - 1) // CHUNK  # 7 (last chunk = 64)

_CACHE = {}


def _build_nc():
    import concourse.bacc as bacc
    import concourse.mybir as mybir
    import concourse.tile as tile

    f32 = mybir.dt.float32
    f32r = mybir.dt.float32r
    mult = mybir.AluOpType.mult
    add = mybir.AluOpType.add
    act_copy = mybir.ActivationFunctionType.Copy

    nc = bacc.Bacc()
    xs = nc.dram_tensor("xs", [NSH, C, H, W], f32, kind="ExternalInput")
    # wd[cb, tap] = diag(wh_tap) for channels cb*128..cb*128+127
    wd = nc.dram_tensor("wd", [CB, 3, P, P], f32, kind="ExternalInput")
    # wv[cb] columns: [wv_m1, wv_0, wv_p1, -wh_m1, -wh_p1]
    wv = nc.dram_tensor("wv", [CB, P, 5], f32, kind="ExternalInput")
    ys = nc.dram_tensor("ys", [NSH, C, H, W], f32, kind="ExternalOutput")

    with tile.TileContext(nc) as tc:
        with tc.tile_pool(name="wp", bufs=1) as wp, \
             tc.tile_pool(name="xp", bufs=3) as xpool, \
             tc.tile_pool(name="ht", bufs=3) as hpool, \
             tc.tile_pool(name="op", bufs=3) as opool, \
             tc.tile_pool(name="ps", bufs=2, space="PSUM") as ppool:

            wdt = []
            wvt = []
            for cb in range(CB):
                t = wp.tile([P, 3 * P], f32r, tag=f"wd{cb}")
                nc.gpsimd.dma_start(
                    t[:].rearrange("p (t q) -> p t q", t=3),
                    wd[cb].rearrange("t p q -> p t q"))
                wdt.append(t)
                v = wp.tile([P, 5], f32, tag=f"wv{cb}")
                nc.sync.dma_start(v[:], wv[cb])
                wvt.append(v)

            for n in range(NSH):
                for cb in range(CB):
                    wvc = wvt[cb]
                    cs = slice(cb * P, (cb + 1) * P)

                    X = xpool.tile([P, XF], f32r)
                    nc.gpsimd.memset(X[:, 0:1].bitcast(f32), 0.0)
                    nc.gpsimd.memset(X[:, XF - 1:XF].bitcast(f32), 0.0)
                    nc.gpsimd.dma_start(X[:, 1:1 + HW], xs[n, cs, :, :])

                    HT = hpool.tile([P, HTF], f32)
                    nc.gpsimd.memset(HT[:, 0:W], 0.0)
                    nc.gpsimd.memset(HT[:, HTF - W:], 0.0)
                    ctr = HT[:, W:W + HW]

                    # H-stage per 4-bank PSUM piece; double-buffered PSUM
                    # pool lets the next piece's matmuls overlap this
                    # piece's copy-out
                    for p0 in range(0, HW, 4 * CHUNK):
                        pn = min(4 * CHUNK, HW - p0)
                        PS = ppool.tile([P, 4 * CHUNK], f32, tag="ps")
                        for c0 in range(0, pn, CHUNK):
                            cn = min(CHUNK, pn - c0)
                            for tap in range(3):
                                nc.tensor.matmul(
                                    PS[:, c0:c0 + cn],
                                    wdt[cb][:, tap * P:(tap + 1) * P],
                                    X[:, p0 + c0 + tap:p0 + c0 + tap + cn],
                                    start=(tap == 0), stop=(tap == 2))
                        nc.scalar.activation(
                            HT[:, W + p0:W + p0 + pn], PS[:, 0:pn], act_copy)

    # wrapped-column fixups (strided, per channel):
                    #   tmp[h][0]  -= wh_m1 * x[h-1][55]   (x[-1][55] := guard X[0] = 0)
                    #   tmp[h][55] -= wh_p1 * x[h+1][0]    (h=55 term is 0: guard X[3137])
                    ctr2 = ctr.rearrange("p (h w) -> p h w", w=W)
                    col0 = ctr2[:, :, 0]
                    col55 = ctr2[:, 0:H - 1, W - 1]
                    # x[h-1][55] = X[56h]; x[h+1][0] = X[1 + 56(h+1)]
                    xg0 = X[:, 0:HW].bitcast(f32).rearrange(
                        "p (h w) -> p h w", w=W)[:, :, 0]
                    xg55 = X[:, 1:1 + HW].bitcast(f32).rearrange(
                        "p (h w) -> p h w", w=W)[:, 1:H, 0]
                    nc.vector.scalar_tensor_tensor(
                        col0, xg0, wvc[:, 3:4], col0, op0=mult, op1=add)
                    nc.vector.scalar_tensor_tensor(
                        col55, xg55, wvc[:, 4:5], col55, op0=mult, op1=add)

                    OUT = opool.tile([P, HW], f32)
                    nc.scalar.activation(OUT[:], ctr, act_copy, scale=wvc[:, 1:2])
                    nc.vector.scalar_tensor_tensor(
                        OUT[:], HT[:, 0:HW], wvc[:, 0:1], OUT[:], op0=mult, op1=add)
                    nc.vector.scalar_tensor_tensor(
                        OUT[:], HT[:, 2 * W:2 * W + HW], wvc[:, 2:3], OUT[:],
                        op0=mult, op1=add)

                    nc.sync.dma_start(
                        ys[n, cs, :, :],
                        OUT[:].rearrange("p (h w) -> p h w", w=W))
    nc.finalize()
    return nc


def _tap_weights(shift):
    """Per-channel 3-tap weights over offsets {-1,0,1} for shift in [-1,1)."""
    f = np.floor(shift)
    t = (shift - f).astype(np.float32)
    assert np.all((f == -1) | (f == 0)), "shift outside [-1,1) unsupported"
    w_m1 = np.where(f == -1, 1 - t, 0).astype(np.float32)
    w_0 = np.where(f == -1, t, 1 - t).astype(np.float32)
    w_p1 = np.where(f == 0, t, 0).astype(np.float32)
    return w_m1, w_0, w_p1


def _host_weights(sp):
    wh_m1, wh_0, wh_p1 = _tap_weights(sp[:, 1])  # beta: W shift
    wv_m1, wv_0, wv_p1 = _tap_weights(sp[:, 0])  # alpha: H shift
    wd = np.zeros((CB, 3, P, P), np.float32)
    for cb in range(CB):
        cs = slice(cb * P, (cb + 1) * P)
        for t, w in enumerate((wh_m1, wh_0, wh_p1)):
            wd[cb, t] = np.diag(w[cs])
    wv = np.stack([wv_m1, wv_0, wv_p1, -wh_m1, -wh_p1], axis=1).astype(np.float32)
    wv = np.ascontiguousarray(wv.reshape(CB, P, 5))
    return np.ascontiguousarray(wd), wv


def _install_trace_shim():
    """Dev-only: register the NTFF profile hook this container's antenv lacks,
    and stub out the artifact upload (zero-egress container)."""
    import sys
    import types

    try:
        from antenv.axon_hooks import get_axon_ntff_profile_hook  # noqa: F401
    except ImportError:
        from trn_agent_boot.trn_boot import _ntff_profile_via_ctypes

        hook = _ntff_profile_via_ctypes("/opt/axon/libaxon_pjrt.so")
        mod = types.ModuleType("antenv.axon_hooks")
        mod.get_axon_ntff_profile_hook = lambda: hook
        mod.set_axon_ntff_profile_hook = lambda h: None
        import antenv

        sys.modules["antenv.axon_hooks"] = mod
        antenv.axon_hooks = mod

    import concourse.bass_utils as bu

    bu.upload_artifacts = lambda tmpdir: tmpdir


def kernel(x, shift_param):
    from concourse.bass_utils import run_bass_kernel_spmd

    x = np.ascontiguousarray(np.asarray(x, dtype=np.float32))
    sp = np.asarray(shift_param, dtype=np.float32)
    assert x.shape == (N, C, H, W)

    wd, wv = _host_weights(sp)

    if "nc" not in _CACHE:
        _CACHE["nc"] = _build_nc()
    nc = _CACHE["nc"]

    in_maps = [{"xs": x[i * NSH:(i + 1) * NSH], "wd": wd, "wv": wv}
               for i in range(NCORES)]
    trace = os.environ.get("ASL_TRACE") == "1"
    if trace:
        _install_trace_shim()
    res = run_bass_kernel_spmd(nc, in_maps, list(range(NCORES)), trace=trace)
    if trace:
        print(f"HW exec time: {res.exec_time_ns} ns")
        _CACHE["last_result"] = res
    out = np.concatenate([r["ys"] for r in res.results], axis=0)
    return out


# revision 15
# speedup vs baseline: 1.1119x; 1.0265x over previous
"""ActiveShiftLayer Trainium2 kernel.

out[n,c,h,w] = bilinear sample of x[n,c, h+alpha_c, w+beta_c], zero outside.

alpha,beta in [-1,1) => floor in {-1,0}; the bilinear sample is a separable
3-tap convolution along W then H with per-channel tap weights:
    tmp[h,w] = sum_dx wh[c,dx] * x[h, w+dx]      (dx in {-1,0,1}, zero pad)
    out[h,w] = sum_dy wv[c,dy] * tmp[h+dy, w]    (dy in {-1,0,1}, zero pad)
Weights are computed on host from shift_param [C,2] and passed as extra
inputs.

Data-parallel over batch (N=32 -> 4 per core); per core 8 tiles of
[128 channels (partitions), 56*56 plane (free dim)].

Per-tile schedule (f32 end-to-end except the H-stage products in float32r):
- contiguous DMA load into X[128, 1+3136+1] (1-elem guard pads)
- H-stage on TensorE: per 512-col chunk, 3 accumulating float32r matmuls
  with diagonal weight matrices (diag applies per-channel tap weight); flat
  taps at offsets {-1,0,+1} wrap across row boundaries, fixed later
- ScalarE copies PSUM -> SBUF HT center (rows 1..56 of a 58-row buffer
  whose first/last rows are zeroed)
- GPSIMD fixes the two wrapped columns: tmp[h][0] -= wh_m1*x[h-1][55],
  tmp[h][55] -= wh_p1*x[h+1][0] (strided 56-elem scalar_tensor_tensor)
- V-stage: ScalarE center tap (activation scale), VectorE outer taps
  (scalar_tensor_tensor accumulate), all exact f32
- contiguous DMA store
"""

import os
import numpy as np

N, C, H, W = 32, 256, 56, 56
NCORES = 8
NSH = N // NCORES  # batches per core
P = 128
CB = C // P        # channel blocks
HW = H * W         # 3136
XF = HW + 2        # X tile free size (guard pad at 0 and HW+1)
HTF = (H + 2) * W  # 3248
CHUNK = 512
NCHUNK = (HW + CHUNK - 1) // CHUNK  # 7 (last chunk = 64)

_CACHE = {}


def _build_nc():
    import concourse.bacc as bacc
    import concourse.mybir as mybir
    import concourse.tile as tile

    f32 = mybir.dt.float32
    f32r = mybir.dt.float32r
    mult = mybir.AluOpType.mult
    add = mybir.AluOpType.add
    act_copy = mybir.ActivationFunctionType.Copy

    nc = bacc.Bacc()
    xs = nc.dram_tensor("xs", [NSH, C, H, W], f32, kind="ExternalInput")
    # wd[cb, tap] = diag(wh_tap) for channels cb*128..cb*128+127
    wd = nc.dram_tensor("wd", [CB, 3, P, P], f32, kind="ExternalInput")
    # wv[cb] columns: [wv_m1, wv_0, wv_p1, -wh_m1, -wh_p1]
    wv = nc.dram_tensor("wv", [CB, P, 5], f32, kind="ExternalInput")
    ys = nc.dram_tensor("ys", [NSH, C, H, W], f32, kind="ExternalOutput")

    with tile.TileContext(nc) as tc:
        with tc.tile_pool(name="wp", bufs=1) as wp, \
             tc.tile_pool(name="xp", bufs=3) as xpool, \
             tc.tile_pool(name="ht", bufs=3) as hpool, \
             tc.tile_pool(name="op", bufs=3) as opool, \
             tc.tile_pool(name="ps", bufs=2, space="PSUM") as ppool:

            wdt = []
            wvt = []
            for cb in range(CB):
                t = wp.tile([P, 3 * P], f32r, tag=f"wd{cb}")
                nc.gpsimd.dma_start(
                    t[:].rearrange("p (t q) -> p t q", t=3),
                    wd[cb].rearrange("t p q -> p t q"))
                wdt.append(t)
                v = wp.tile([P, 5], f32, tag=f"wv{cb}")
                nc.sync.dma_start(v[:], wv[cb])
                wvt.append(v)

            for n in range(NSH):
                for cb in range(CB):
                    wvc = wvt[cb]
                    cs = slice(cb * P, (cb + 1) * P)

                    # X holds one zero guard row above and below the plane so
                    # the V-stage taps (row shifts) read true zeros
                    X = xpool.tile([P, W + HW + W], f32r)
                    nc.gpsimd.memset(X[:, 0:W].bitcast(f32), 0.0)
                    nc.gpsimd.memset(X[:, W + HW:].bitcast(f32), 0.0)
                    nc.gpsimd.dma_start(X[:, W:W + HW], xs[n, cs, :, :])

                    # VT gets one zero guard element on each side for the
                    # H-stage taps
                    VT = hpool.tile([P, 1 + HW + 1], f32)
                    nc.vector.memset(VT[:, 0:1], 0.0)
                    nc.vector.memset(VT[:, 1 + HW:], 0.0)
                    ctr = VT[:, 1:1 + HW]

                    # V-stage on TensorE: per 4-bank PSUM piece, 3
                    # accumulating diag matmuls (taps at row offsets
                    # -56/0/+56); double-buffered PSUM pool overlaps the
                    # next piece's matmuls with this piece's copy-out
                    for p0 in range(0, HW, 4 * CHUNK):
                        pn = min(4 * CHUNK, HW - p0)
                        PS = ppool.tile([P, 4 * CHUNK], f32, tag="ps")
                        for c0 in range(0, pn, CHUNK):
                            cn = min(CHUNK, pn - c0)
                            for tap in range(3):
                                o = p0 + c0 + tap * W
                                nc.tensor.matmul(
                                    PS[:, c0:c0 + cn],
                                    wdt[cb][:, tap * P:(tap + 1) * P],
                                    X[:, o:o + cn],
                                    start=(tap == 0), stop=(tap == 2))
                        nc.scalar.activation(
                            VT[:, 1 + p0:1 + p0 + pn], PS[:, 0:pn], act_copy)

                    # H-stage: center tap on ScalarE, outer taps on VectorE.
                    # Flat taps at -1/+1 wrap across row boundaries; the two
                    # wrapped columns are corrected at the end on OUT.
                    OUT = opool.tile([P, HW], f32)
                    nc.scalar.activation(OUT[:], ctr, act_copy, scale=wvc[:, 1:2])
                    nc.vector.scalar_tensor_tensor(
                        OUT[:], VT[:, 0:HW], wvc[:, 0:1], OUT[:], op0=mult, op1=add)
                    nc.vector.scalar_tensor_tensor(
                        OUT[:], VT[:, 2:2 + HW], wvc[:, 2:3], OUT[:],
                        op0=mult, op1=add)

                    # out[h][0]  -= wh_m1 * vt[h-1][55]  (vt[-1][55] := VT[0] guard)
                    # out[h][55] -= wh_p1 * vt[h+1][0]   (h=55 term is 0: VT[3137] guard)
                    out2 = OUT[:].rearrange("p (h w) -> p h w", w=W)
                    col0 = out2[:, :, 0]
                    col55 = out2[:, 0:H - 1, W - 1]
                    vg0 = VT[:, 0:HW].rearrange("p (h w) -> p h w", w=W)[:, :, 0]
                    vg55 = VT[:, 1:1 + HW].rearrange(
                        "p (h w) -> p h w", w=W)[:, 1:H, 0]
                    nc.vector.scalar_tensor_tensor(
                        col0, vg0, wvc[:, 3:4], col0, op0=mult, op1=add)
                    nc.vector.scalar_tensor_tensor(
                        col55, vg55, wvc[:, 4:5], col55, op0=mult, op1=add)

                    nc.sync.dma_start(
                        ys[n, cs, :, :],
                        OUT[:].rearrange("p (h w) -> p h w", w=W))
    nc.finalize()
    return nc


def _tap_weights(shift):
    """Per-channel 3-tap weights over offsets {-1,0,1} for shift in [-1,1)."""
    f = np.floor(shift)
    t = (shift - f).astype(np.float32)
    assert np.all((f == -1) | (f == 0)), "shift outside [-1,1) unsupported"
    w_m1 = np.where(f == -1, 1 - t, 0).astype(np.float32)
    w_0 = np.where(f == -1, t, 1 - t).astype(np.float32)
    w_p1 = np.where(f == 0, t, 0).astype(np.float32)
    return w_m1, w_0, w_p1


def _host_weights(sp):
    wh_m1, wh_0, wh_p1 = _tap_weights(sp[:, 1])  # beta: W shift
    wv_m1, wv_0, wv_p1 = _tap_weights(sp[:, 0])  # alpha: H shift
    # V-stage taps run on TensorE as diagonal matrices
    wd = np.zeros((CB, 3, P, P), np.float32)
    for cb in range(CB):
        cs = slice(cb * P, (cb + 1) * P)
        for t, w in enumerate((wv_m1, wv_0, wv_p1)):
            wd[cb, t] = np.diag(w[cs])
    # H-stage per-partition scalars + negated outer taps for wrap fixups
    wv = np.stack([wh_m1, wh_0, wh_p1, -wh_m1, -wh_p1], axis=1).astype(np.float32)
    wv = np.ascontiguousarray(wv.reshape(CB, P, 5))
    return np.ascontiguousarray(wd), wv


def _install_trace_shim():
    """Dev-only: register the NTFF profile hook this container's antenv lacks,
    and stub out the artifact upload (zero-egress container)."""
    import sys
    import types

    try:
        from antenv.axon_hooks import get_axon_ntff_profile_hook  # noqa: F401
    except ImportError:
        from trn_agent_boot.trn_boot import _ntff_profile_via_ctypes

        hook = _ntff_profile_via_ctypes("/opt/axon/libaxon_pjrt.so")
        mod = types.ModuleType("antenv.axon_hooks")
        mod.get_axon_ntff_profile_hook = lambda: hook
        mod.set_axon_ntff_profile_hook = lambda h: None
        import antenv

        sys.modules["antenv.axon_hooks"] = mod
        antenv.axon_hooks = mod

    import concourse.bass_utils as bu

    bu.upload_artifacts = lambda tmpdir: tmpdir


def kernel(x, shift_param):
    from concourse.bass_utils import run_bass_kernel_spmd

    x = np.ascontiguousarray(np.asarray(x, dtype=np.float32))
    sp = np.asarray(shift_param, dtype=np.float32)
    assert x.shape == (N, C, H, W)

    wd, wv = _host_weights(sp)

    if "nc" not in _CACHE:
        _CACHE["nc"] = _build_nc()
    nc = _CACHE["nc"]

    in_maps = [{"xs": x[i * NSH:(i + 1) * NSH], "wd": wd, "wv": wv}
               for i in range(NCORES)]
    trace = os.environ.get("ASL_TRACE") == "1"
    if trace:
        _install_trace_shim()
    res = run_bass_kernel_spmd(nc, in_maps, list(range(NCORES)), trace=trace)
    if trace:
        print(f"HW exec time: {res.exec_time_ns} ns")
        _CACHE["last_result"] = res
    out = np.concatenate([r["ys"] for r in res.results], axis=0)
    return out


# revision 16
# speedup vs baseline: 1.1434x; 1.0284x over previous
"""ActiveShiftLayer Trainium2 kernel.

out[n,c,h,w] = bilinear sample of x[n,c, h+alpha_c, w+beta_c], zero outside.

alpha,beta in [-1,1) => floor in {-1,0}; the bilinear sample is a separable
3-tap convolution along W then H with per-channel tap weights:
    tmp[h,w] = sum_dx wh[c,dx] * x[h, w+dx]      (dx in {-1,0,1}, zero pad)
    out[h,w] = sum_dy wv[c,dy] * tmp[h+dy, w]    (dy in {-1,0,1}, zero pad)
Weights are computed on host from shift_param [C,2] and passed as extra
inputs.

Data-parallel over batch (N=32 -> 4 per core); per core 8 tiles of
[128 channels (partitions), 56*56 plane (free dim)].

Per-tile schedule (f32 end-to-end except the H-stage products in float32r):
- contiguous DMA load into X[128, 1+3136+1] (1-elem guard pads)
- H-stage on TensorE: per 512-col chunk, 3 accumulating float32r matmuls
  with diagonal weight matrices (diag applies per-channel tap weight); flat
  taps at offsets {-1,0,+1} wrap across row boundaries, fixed later
- ScalarE copies PSUM -> SBUF HT center (rows 1..56 of a 58-row buffer
  whose first/last rows are zeroed)
- GPSIMD fixes the two wrapped columns: tmp[h][0] -= wh_m1*x[h-1][55],
  tmp[h][55] -= wh_p1*x[h+1][0] (strided 56-elem scalar_tensor_tensor)
- V-stage: ScalarE center tap (activation scale), VectorE outer taps
  (scalar_tensor_tensor accumulate), all exact f32
- contiguous DMA store
"""

import os
import numpy as np

N, C, H, W = 32, 256, 56, 56
NCORES = 8
NSH = N // NCORES  # batches per core
P = 128
CB = C // P        # channel blocks
HW = H * W         # 3136
XF = HW + 2        # X tile free size (guard pad at 0 and HW+1)
HTF = (H + 2) * W  # 3248
CHUNK = 512
NCHUNK = (HW + CHUNK - 1) // CHUNK  # 7 (last chunk = 64)

_CACHE = {}


def _build_nc():
    import concourse.bacc as bacc
    import concourse.mybir as mybir
    import concourse.tile as tile

    f32 = mybir.dt.float32
    f32r = mybir.dt.float32r
    mult = mybir.AluOpType.mult
    add = mybir.AluOpType.add
    act_copy = mybir.ActivationFunctionType.Copy

    nc = bacc.Bacc()
    xs = nc.dram_tensor("xs", [NSH, C, H, W], f32r, kind="ExternalInput")
    # wd[cb, tap] = diag(wh_tap) for channels cb*128..cb*128+127
    wd = nc.dram_tensor("wd", [CB, 3, P, P], f32r, kind="ExternalInput")
    # wv[cb] columns: [wv_m1, wv_0, wv_p1, -wh_m1, -wh_p1]
    wv = nc.dram_tensor("wv", [CB, P, 5], f32, kind="ExternalInput")
    ys = nc.dram_tensor("ys", [NSH, C, H, W], f32, kind="ExternalOutput")

    with tile.TileContext(nc) as tc:
        with tc.tile_pool(name="wp", bufs=1) as wp, \
             tc.tile_pool(name="xp", bufs=4) as xpool, \
             tc.tile_pool(name="ht", bufs=4) as hpool, \
             tc.tile_pool(name="op", bufs=4) as opool, \
             tc.tile_pool(name="ps", bufs=2, space="PSUM") as ppool:

            wdt = []
            wvt = []
            for cb in range(CB):
                t = wp.tile([P, 3 * P], f32r, tag=f"wd{cb}")
                nc.sync.dma_start(
                    t[:].rearrange("p (t q) -> p t q", t=3),
                    wd[cb].rearrange("t p q -> p t q"))
                wdt.append(t)
                v = wp.tile([P, 5], f32, tag=f"wv{cb}")
                nc.sync.dma_start(v[:], wv[cb])
                wvt.append(v)

            for n in range(NSH):
                for cb in range(CB):
                    wvc = wvt[cb]
                    cs = slice(cb * P, (cb + 1) * P)

                    # X holds one zero guard row above and below the plane so
                    # the V-stage taps (row shifts) read true zeros
                    X = xpool.tile([P, 3264], f32r)
                    nc.gpsimd.memset(X[:, 0:W].bitcast(f32), 0.0)
                    nc.gpsimd.memset(X[:, W + HW:W + HW + W].bitcast(f32), 0.0)
                    nc.sync.dma_start(X[:, W:W + HW], xs[n, cs, :, :])

                    # VT gets one zero guard element on each side for the
                    # H-stage taps
                    VT = hpool.tile([P, 3200], f32)
                    nc.vector.memset(VT[:, 0:1], 0.0)
                    nc.vector.memset(VT[:, 1 + HW:2 + HW], 0.0)
                    ctr = VT[:, 1:1 + HW]

                    # V-stage on TensorE: per 4-bank PSUM piece, 3
                    # accumulating diag matmuls (taps at row offsets
                    # -56/0/+56); double-buffered PSUM pool overlaps the
                    # next piece's matmuls with this piece's copy-out
                    for p0 in range(0, HW, 4 * CHUNK):
                        pn = min(4 * CHUNK, HW - p0)
                        PS = ppool.tile([P, 4 * CHUNK], f32, tag="ps")
                        for c0 in range(0, pn, CHUNK):
                            cn = min(CHUNK, pn - c0)
                            for tap in range(3):
                                o = p0 + c0 + tap * W
                                nc.tensor.matmul(
                                    PS[:, c0:c0 + cn],
                                    wdt[cb][:, tap * P:(tap + 1) * P],
                                    X[:, o:o + cn],
                                    start=(tap == 0), stop=(tap == 2))
                        nc.scalar.activation(
                            VT[:, 1 + p0:1 + p0 + pn], PS[:, 0:pn], act_copy)

                    # H-stage: center tap on ScalarE, outer taps on VectorE.
                    # Flat taps at -1/+1 wrap across row boundaries; the two
                    # wrapped columns are corrected at the end on OUT.
                    OUT = opool.tile([P, HW], f32)
                    nc.scalar.activation(OUT[:], ctr, act_copy, scale=wvc[:, 1:2])
                    nc.vector.scalar_tensor_tensor(
                        OUT[:], VT[:, 0:HW], wvc[:, 0:1], OUT[:], op0=mult, op1=add)
                    nc.vector.scalar_tensor_tensor(
                        OUT[:], VT[:, 2:2 + HW], wvc[:, 2:3], OUT[:],
                        op0=mult, op1=add)

                    # out[h][0]  -= wh_m1 * vt[h-1][55]  (vt[-1][55] := VT[0] guard)
                    # out[h][55] -= wh_p1 * vt[h+1][0]   (h=55 term is 0: VT[3137] guard)
                    out2 = OUT[:].rearrange("p (h w) -> p h w", w=W)
                    col0 = out2[:, :, 0]
                    col55 = out2[:, 0:H - 1, W - 1]
                    vg0 = VT[:, 0:HW].rearrange("p (h w) -> p h w", w=W)[:, :, 0]
                    vg55 = VT[:, 1:1 + HW].rearrange(
                        "p (h w) -> p h w", w=W)[:, 1:H, 0]
                    nc.vector.scalar_tensor_tensor(
                        col0, vg0, wvc[:, 3:4], col0, op0=mult, op1=add)
                    nc.vector.scalar_tensor_tensor(
                        col55, vg55, wvc[:, 4:5], col55, op0=mult, op1=add)

                    nc.sync.dma_start(
                        ys[n, cs, :, :],
                        OUT[:].rearrange("p (h w) -> p h w", w=W))
    nc.finalize()
    return nc


def _tap_weights(shift):
    """Per-channel 3-tap weights over offsets {-1,0,1} for shift in [-1,1)."""
    f = np.floor(shift)
    t = (shift - f).astype(np.float32)
    assert np.all((f == -1) | (f == 0)), "shift outside [-1,1) unsupported"
    w_m1 = np.where(f == -1, 1 - t, 0).astype(np.float32)
    w_0 = np.where(f == -1, t, 1 - t).astype(np.float32)
    w_p1 = np.where(f == 0, t, 0).astype(np.float32)
    return w_m1, w_0, w_p1


def _host_weights(sp):
    wh_m1, wh_0, wh_p1 = _tap_weights(sp[:, 1])  # beta: W shift
    wv_m1, wv_0, wv_p1 = _tap_weights(sp[:, 0])  # alpha: H shift
    # V-stage taps run on TensorE as diagonal matrices
    wd = np.zeros((CB, 3, P, P), np.float32)
    for cb in range(CB):
        cs = slice(cb * P, (cb + 1) * P)
        for t, w in enumerate((wv_m1, wv_0, wv_p1)):
            wd[cb, t] = np.diag(w[cs])
    # H-stage per-partition scalars + negated outer taps for wrap fixups
    wv = np.stack([wh_m1, wh_0, wh_p1, -wh_m1, -wh_p1], axis=1).astype(np.float32)
    wv = np.ascontiguousarray(wv.reshape(CB, P, 5))
    return np.ascontiguousarray(wd), wv


def _install_trace_shim():
    """Dev-only: register the NTFF profile hook this container's antenv lacks,
    and stub out the artifact upload (zero-egress container)."""
    import sys
    import types

    try:
        from antenv.axon_hooks import get_axon_ntff_profile_hook  # noqa: F401
    except ImportError:
        from trn_agent_boot.trn_boot import _ntff_profile_via_ctypes

        hook = _ntff_profile_via_ctypes("/opt/axon/libaxon_pjrt.so")
        mod = types.ModuleType("antenv.axon_hooks")
        mod.get_axon_ntff_profile_hook = lambda: hook
        mod.set_axon_ntff_profile_hook = lambda h: None
        import antenv

        sys.modules["antenv.axon_hooks"] = mod
        antenv.axon_hooks = mod

    import concourse.bass_utils as bu

    bu.upload_artifacts = lambda tmpdir: tmpdir


def kernel(x, shift_param):
    from concourse.bass_utils import run_bass_kernel_spmd

    x = np.ascontiguousarray(np.asarray(x, dtype=np.float32))
    sp = np.asarray(shift_param, dtype=np.float32)
    assert x.shape == (N, C, H, W)

    wd, wv = _host_weights(sp)

    if "nc" not in _CACHE:
        _CACHE["nc"] = _build_nc()
    nc = _CACHE["nc"]

    in_maps = [{"xs": x[i * NSH:(i + 1) * NSH], "wd": wd, "wv": wv}
               for i in range(NCORES)]
    trace = os.environ.get("ASL_TRACE") == "1"
    if trace:
        _install_trace_shim()
    res = run_bass_kernel_spmd(nc, in_maps, list(range(NCORES)), trace=trace)
    if trace:
        print(f"HW exec time: {res.exec_time_ns} ns")
        _CACHE["last_result"] = res
    out = np.concatenate([r["ys"] for r in res.results], axis=0)
    return out


# revision 18
# speedup vs baseline: 1.1915x; 1.0420x over previous
"""ActiveShiftLayer Trainium2 kernel.

out[n,c,h,w] = bilinear sample of x[n,c, h+alpha_c, w+beta_c], zero outside.

alpha,beta in [-1,1) => floor in {-1,0}; the bilinear sample is a separable
3-tap convolution along W then H with per-channel tap weights:
    tmp[h,w] = sum_dx wh[c,dx] * x[h, w+dx]      (dx in {-1,0,1}, zero pad)
    out[h,w] = sum_dy wv[c,dy] * tmp[h+dy, w]    (dy in {-1,0,1}, zero pad)
Weights are computed on host from shift_param [C,2] and passed as extra
inputs.

Data-parallel over batch (N=32 -> 4 per core); per core 8 tiles of
[128 channels (partitions), 56*56 plane (free dim)].

Per-tile schedule (f32 end-to-end except the H-stage products in float32r):
- contiguous DMA load into X[128, 1+3136+1] (1-elem guard pads)
- H-stage on TensorE: per 512-col chunk, 3 accumulating float32r matmuls
  with diagonal weight matrices (diag applies per-channel tap weight); flat
  taps at offsets {-1,0,+1} wrap across row boundaries, fixed later
- ScalarE copies PSUM -> SBUF HT center (rows 1..56 of a 58-row buffer
  whose first/last rows are zeroed)
- GPSIMD fixes the two wrapped columns: tmp[h][0] -= wh_m1*x[h-1][55],
  tmp[h][55] -= wh_p1*x[h+1][0] (strided 56-elem scalar_tensor_tensor)
- V-stage: ScalarE center tap (activation scale), VectorE outer taps
  (scalar_tensor_tensor accumulate), all exact f32
- contiguous DMA store
"""

import os
import numpy as np

N, C, H, W = 32, 256, 56, 56
NCORES = 8
NSH = N // NCORES  # batches per core
P = 128
CB = C // P        # channel blocks
HW = H * W         # 3136
XF = HW + 2        # X tile free size (guard pad at 0 and HW+1)
HTF = (H + 2) * W  # 3248
CHUNK = 512
NCHUNK = (HW + CHUNK - 1) // CHUNK  # 7 (last chunk = 64)

_CACHE = {}


def _build_nc():
    import concourse.bacc as bacc
    import concourse.mybir as mybir
    import concourse.tile as tile

    f32 = mybir.dt.float32
    f32r = mybir.dt.float32r
    mult = mybir.AluOpType.mult
    add = mybir.AluOpType.add
    act_copy = mybir.ActivationFunctionType.Copy

    nc = bacc.Bacc()
    xs = nc.dram_tensor("xs", [NSH, C, H, W], f32r, kind="ExternalInput")
    # wd[cb, tap] = diag(wh_tap) for channels cb*128..cb*128+127
    wd = nc.dram_tensor("wd", [CB, 3, P, P], f32r, kind="ExternalInput")
    # wv[cb] columns: [wv_m1, wv_0, wv_p1, -wh_m1, -wh_p1]
    wv = nc.dram_tensor("wv", [CB, P, 5], f32, kind="ExternalInput")
    ys = nc.dram_tensor("ys", [NSH, C, H, W], f32, kind="ExternalOutput")

    with tile.TileContext(nc) as tc:
        with tc.tile_pool(name="wp", bufs=1) as wp, \
             tc.tile_pool(name="xp", bufs=4) as xpool, \
             tc.tile_pool(name="ht", bufs=4) as hpool, \
             tc.tile_pool(name="op", bufs=4) as opool, \
             tc.tile_pool(name="ps", bufs=2, space="PSUM") as ppool:

            wdt = []
            wvt = []
            for cb in range(CB):
                t = wp.tile([P, 3 * P], f32r, tag=f"wd{cb}")
                nc.sync.dma_start(
                    t[:].rearrange("p (t q) -> p t q", t=3),
                    wd[cb].rearrange("t p q -> p t q"))
                wdt.append(t)
                v = wp.tile([P, 5], f32, tag=f"wv{cb}")
                nc.sync.dma_start(v[:], wv[cb])
                wvt.append(v)

            for n in range(NSH):
                for cb in range(CB):
                    wvc = wvt[cb]
                    cs = slice(cb * P, (cb + 1) * P)

                    # X holds one zero guard row above and below the plane so
                    # the V-stage taps (row shifts) read true zeros
                    X = xpool.tile([P, 3264], f32r)
                    nc.gpsimd.memset(X[:, 0:W].bitcast(f32), 0.0)
                    nc.gpsimd.memset(X[:, W + HW:W + HW + W].bitcast(f32), 0.0)
                    nc.sync.dma_start(X[:, W:W + HW], xs[n, cs, :, :])

                    VT = hpool.tile([P, 3200], f32)
                    ctr = VT[:, 0:HW]

                    # V-stage on TensorE: per 4-bank PSUM piece, 3
                    # accumulating diag matmuls (taps at row offsets
                    # -56/0/+56); double-buffered PSUM pool overlaps the
                    # next piece's matmuls with this piece's copy-out
                    for p0 in range(0, HW, 4 * CHUNK):
                        pn = min(4 * CHUNK, HW - p0)
                        PS = ppool.tile([P, 4 * CHUNK], f32, tag="ps")
                        for c0 in range(0, pn, CHUNK):
                            cn = min(CHUNK, pn - c0)
                            for tap in range(3):
                                o = p0 + c0 + tap * W
                                nc.tensor.matmul(
                                    PS[:, c0:c0 + cn],
                                    wdt[cb][:, tap * P:(tap + 1) * P],
                                    X[:, o:o + cn],
                                    start=(tap == 0), stop=(tap == 2))
                        nc.scalar.activation(
                            VT[:, p0:p0 + pn], PS[:, 0:pn], act_copy)

                    # H-stage: center tap on ScalarE, outer taps on VectorE
                    # as 2D-AP scalar_tensor_tensor that EXCLUDE the column
                    # where the shifted sample is out of bounds (its true
                    # contribution is zero), so no wrap fixups are needed.
                    OUT = opool.tile([P, HW], f32)
                    nc.scalar.activation(OUT[:], ctr, act_copy, scale=wvc[:, 1:2])
                    o2 = OUT[:].rearrange("p (h w) -> p h w", w=W)
                    v2 = ctr.rearrange("p (h w) -> p h w", w=W)
                    nc.vector.scalar_tensor_tensor(
                        o2[:, :, 1:W], v2[:, :, 0:W - 1], wvc[:, 0:1],
                        o2[:, :, 1:W], op0=mult, op1=add)
                    nc.vector.scalar_tensor_tensor(
                        o2[:, :, 0:W - 1], v2[:, :, 1:W], wvc[:, 2:3],
                        o2[:, :, 0:W - 1], op0=mult, op1=add)

                    nc.sync.dma_start(
                        ys[n, cs, :, :],
                        OUT[:].rearrange("p (h w) -> p h w", w=W))
    nc.finalize()
    return nc


def _tap_weights(shift):
    """Per-channel 3-tap weights over offsets {-1,0,1} for shift in [-1,1)."""
    f = np.floor(shift)
    t = (shift - f).astype(np.float32)
    assert np.all((f == -1) | (f == 0)), "shift outside [-1,1) unsupported"
    w_m1 = np.where(f == -1, 1 - t, 0).astype(np.float32)
    w_0 = np.where(f == -1, t, 1 - t).astype(np.float32)
    w_p1 = np.where(f == 0, t, 0).astype(np.float32)
    return w_m1, w_0, w_p1


def _host_weights(sp):
    wh_m1, wh_0, wh_p1 = _tap_weights(sp[:, 1])  # beta: W shift
    wv_m1, wv_0, wv_p1 = _tap_weights(sp[:, 0])  # alpha: H shift
    # V-stage taps run on TensorE as diagonal matrices
    wd = np.zeros((CB, 3, P, P), np.float32)
    for cb in range(CB):
        cs = slice(cb * P, (cb + 1) * P)
        for t, w in enumerate((wv_m1, wv_0, wv_p1)):
            wd[cb, t] = np.diag(w[cs])
    # H-stage per-partition scalars + negated outer taps for wrap fixups
    wv = np.stack([wh_m1, wh_0, wh_p1, -wh_m1, -wh_p1], axis=1).astype(np.float32)
    wv = np.ascontiguousarray(wv.reshape(CB, P, 5))
    return np.ascontiguousarray(wd), wv


def _install_trace_shim():
    """Dev-only: register the NTFF profile hook this container's antenv lacks,
    and stub out the artifact upload (zero-egress container)."""
    import sys
    import types

    try:
        from antenv.axon_hooks import get_axon_ntff_profile_hook  # noqa: F401
    except ImportError:
        from trn_agent_boot.trn_boot import _ntff_profile_via_ctypes

        hook = _ntff_profile_via_ctypes("/opt/axon/libaxon_pjrt.so")
        mod = types.ModuleType("antenv.axon_hooks")
        mod.get_axon_ntff_profile_hook = lambda: hook
        mod.set_axon_ntff_profile_hook = lambda h: None
        import antenv

        sys.modules["antenv.axon_hooks"] = mod
        antenv.axon_hooks = mod

    import concourse.bass_utils as bu

    bu.upload_artifacts = lambda tmpdir: tmpdir


def kernel(x, shift_param):
    from concourse.bass_utils import run_bass_kernel_spmd

    x = np.ascontiguousarray(np.asarray(x, dtype=np.float32))
    sp = np.asarray(shift_param, dtype=np.float32)
    assert x.shape == (N, C, H, W)

    wd, wv = _host_weights(sp)

    if "nc" not in _CACHE:
        _CACHE["nc"] = _build_nc()
    nc = _CACHE["nc"]

    in_maps = [{"xs": x[i * NSH:(i + 1) * NSH], "wd": wd, "wv": wv}
               for i in range(NCORES)]
    trace = os.environ.get("ASL_TRACE") == "1"
    if trace:
        _install_trace_shim()
    res = run_bass_kernel_spmd(nc, in_maps, list(range(NCORES)), trace=trace)
    if trace:
        print(f"HW exec time: {res.exec_time_ns} ns")
        _CACHE["last_result"] = res
    out = np.concatenate([r["ys"] for r in res.results], axis=0)
    return out
